# revision 17
# baseline (speedup 1.0000x reference)
"""Trainium2 Bass kernel for a pre-LN causal transformer block.

Sharding: data-parallel over (batch, sequence-half) -> 8 uniform SPMD shards.
Each core handles 1024 queries of one batch against that batch's 2048 keys,
with causality enforced by a host-supplied multiplicative mask applied after
exp (so the SPMD program is identical on every core).

Layout: all activations are E-major ("transposed", [E, tokens]) so every
matmul contraction lands on the partition dim with zero on-device transposes.
LayerNorm is folded into the projections:
    h = (x - mu) * r * gamma + beta;  h @ W
      = r_t * (x @ (gamma*W)) + (-mu_t) * colsum(gamma*W) + beta @ W
so the device computes raw = (gamma*W)^T @ xT, adds the rank-2 correction
( rows (-mu, 1/r) x rows (colsum, beta@W) ) via a K=128-padded matmul into
PSUM, and multiplies by r_t during the PSUM->SBUF copy.

Matmul inputs are bf16 (full PE rate), accumulation f32 in PSUM, softmax and
residuals f32.
"""

import math
from contextlib import ExitStack

import numpy as np
import ml_dtypes

import concourse.bass as bass
import concourse.tile as tile
from concourse import bacc
from concourse.tile import add_dep_helper
from concourse import mybir
from concourse.bass_utils import run_bass_kernel_spmd

F32 = mybir.dt.float32
BF16 = mybir.dt.bfloat16
AF = mybir.ActivationFunctionType

# Full-size problem dims (hardcoded; the harness provides x of this shape).
DIMS = dict(B=4, C=2048, E=1024, H=16, D=64, FF=4096, EPS=1e-5)
N_CORES = 8
P = 128


def _ceil_div(a, b):
    return (a + b - 1) // b


def build_program(dims):
    """Build the SPMD Bass program. Returns nc."""
    B = dims["B"]
    C = dims["C"]
    E = dims["E"]
    H = dims["H"]
    D = dims["D"]
    FF = dims["FF"]
    EPS = dims["EPS"]

    TKV = C                      # kv tokens per core
    TQ = B * C // N_CORES        # query tokens per core
    ES = E // P                  # E subtiles (contraction)
    FS = FF // P                 # FF subtiles
    HPAIRS = H // 2              # head pairs (Qt/Kt partition packing)
    NQUAD = H // 4               # head quads (V projection batches)
    KT = TKV // P                # key tiles
    QTT = TQ // P                # query token tiles
    QC = _ceil_div(TQ, 512)      # 512-wide query chunks
    QW = TQ // QC                # query chunk width (512 normally)
    KVC = _ceil_div(TKV, 512)    # 512-wide kv chunks
    KVW = TKV // KVC
    assert D == 64 and E == H * D

    nc = bacc.Bacc("TRN2", target_bir_lowering=False, debug=False)

    # ---- DRAM I/O ----
    xkv_d = nc.dram_tensor("xkv", [E, TKV], BF16, kind="ExternalInput")
    xq_d = nc.dram_tensor("xq", [E, TQ], BF16, kind="ExternalInput")
    xqres_d = nc.dram_tensor("xqres", [E, TQ], F32, kind="ExternalInput")
    mask_d = nc.dram_tensor("maskT", [TKV, TQ], BF16, kind="ExternalInput")
    wq_d = nc.dram_tensor("wq", [E, E], BF16, kind="ExternalInput")
    wk_d = nc.dram_tensor("wk", [E, E], BF16, kind="ExternalInput")
    wv_d = nc.dram_tensor("wv", [E, E], BF16, kind="ExternalInput")
    wo_d = nc.dram_tensor("wo", [E, E], BF16, kind="ExternalInput")
    w1_d = nc.dram_tensor("w1", [E, FF], BF16, kind="ExternalInput")
    w2_d = nc.dram_tensor("w2", [FF, E], BF16, kind="ExternalInput")
    # fold tensors: row0 = colsum(W'), row1 = beta @ W  (padded to 128 rows)
    qf_d = nc.dram_tensor("qfold", [P, E], BF16, kind="ExternalInput")
    kf_d = nc.dram_tensor("kfold", [P, E], BF16, kind="ExternalInput")
    vf_d = nc.dram_tensor("vfold", [P, E], BF16, kind="ExternalInput")
    w1f_d = nc.dram_tensor("w1fold", [P, FF], BF16, kind="ExternalInput")
    b1f_d = nc.dram_tensor("b1f", [P, FS], F32, kind="ExternalInput")
    b2f_d = nc.dram_tensor("b2f", [P, ES], F32, kind="ExternalInput")
    out_d = nc.dram_tensor("outT", [E, TQ], F32, kind="ExternalOutput")

    xkv3 = xkv_d.rearrange("(s p) t -> p s t", p=P)
    xq3 = xq_d.rearrange("(s p) t -> p s t", p=P)
    xqres3 = xqres_d.rearrange("(s p) t -> p s t", p=P)
    mask3 = mask_d.rearrange("(s p) t -> p s t", p=P)
    out3 = out_d.rearrange("(s p) t -> p s t", p=P)

    with tile.TileContext(nc) as tc, ExitStack() as ctx:
        perm = ctx.enter_context(tc.tile_pool(name="perm", bufs=1))
        tmp = ctx.enter_context(tc.tile_pool(name="tmp", bufs=2))
        wstream = ctx.enter_context(tc.tile_pool(name="wstream", bufs=2))
        ps = ctx.enter_context(tc.tile_pool(name="ps", bufs=2, space="PSUM"))
        pso = ctx.enter_context(tc.tile_pool(name="pso", bufs=2, space="PSUM"))
        dpool = ctx.enter_context(tc.tile_pool(name="dpool", bufs=2,
                                               space="DRAM"))

        def bcast_rows(dst, srcrow, nrows, width):
            """Broadcast a [1, width] sbuf row to [nrows, width] via DRAM."""
            row_d = dpool.tile([1, width], srcrow.dtype, tag="row_d")
            nc.sync.dma_start(row_d, srcrow)
            bsrc = bass.AP(tensor=row_d.tensor, offset=row_d.offset,
                           ap=[[0, nrows]] + row_d.ap[1:])
            nc.gpsimd.dma_start(dst, bsrc)

        ones_bf = perm.tile([P, 1], BF16, tag="ones_bf")
        nc.vector.memset(ones_bf, 1.0)

        # ACT LUT table management: Exp and Gelu live in different hardware
        # tables, and walrus's table-switch rides the first activation using
        # the new table -- which then only supports a single sync wait. Emit
        # zero-dependency dummy activations to carry each switch; order them
        # on the ACT stream with same-engine dep edges (no semaphores).
        scr_in = perm.tile([1, 8], F32, tag="scr_in")
        nc.vector.memset(scr_in, 1.0)
        scr_out = perm.tile([1, 8], F32, tag="scr_out")
        dummy_exp = nc.scalar.activation(scr_out, scr_in, AF.Exp)
        act_exp_insts = []

        # Warm up every DVE / PE opcode on scratch so first-use config
        # loads don't ride real (multi-wait) instructions.
        A = mybir.AluOpType
        nc.vector.tensor_copy(scr_out, scr_in)
        nc.vector.tensor_mul(scr_out, scr_in, scr_in)
        nc.vector.tensor_add(scr_out, scr_in, scr_in)
        nc.vector.tensor_sub(scr_out, scr_in, scr_in)
        nc.vector.tensor_scalar(scr_out, scr_in, 0.5, 0.5, A.mult, A.add)
        nc.vector.tensor_scalar_mul(scr_out, scr_in, 0.5)
        nc.vector.tensor_scalar_add(scr_out, scr_in, 0.5)
        nc.vector.reciprocal(scr_out, scr_in)
        nc.vector.scalar_tensor_tensor(scr_out, scr_in, 0.5, scr_in,
                                       A.add, A.add)
        scr_bf = perm.tile([1, 8], BF16, tag="scr_bf")
        nc.vector.memset(scr_bf, 1.0)
        nc.vector.tensor_mul(scr_bf, scr_bf, scr_bf)
        scr_ps = ps.tile([P, 1024], F32, tag="ps")
        nc.tensor.matmul(scr_ps[0:8, 0:8], scr_bf[0:1, 0:8],
                         scr_bf[0:1, 0:8], start=True, stop=True)
        nc.vector.tensor_copy(scr_out, scr_ps[0:1, 0:8])
        b1f_sb = perm.tile([P, FS], F32, tag="b1f")
        nc.sync.dma_start(b1f_sb, b1f_d[:, :])
        b2f_sb = perm.tile([P, ES], F32, tag="b2f")
        nc.sync.dma_start(b2f_sb, b2f_d[:, :])
        hidden = perm.tile([P, HPAIRS, TQ], BF16, tag="hidden")

        # ---------- LN statistics (per token, over E) ----------
        # foldrow[0] = -mu, foldrow[1] = 1/r = sqrt(var+eps); rows 2.. = 0.
        # a_bcast = r broadcast to all 128 partitions (bf16).
        def ln_stats(src_sb, ntok, foldrow, a_bcast, a_colT=None):
            nchunk = _ceil_div(ntok, 512)
            w = ntok // nchunk
            for c in range(nchunk):
                sl = slice(c * w, (c + 1) * w)
                pst = ps.tile([P, 1024], F32, tag="ps")
                psum_s = pst[0:1, 0:w]
                psum_q = pst[0:1, 512:512 + w]
                for s in range(ES):
                    nc.tensor.matmul(psum_s, ones_bf, src_sb[:, s, sl],
                                     start=(s == 0), stop=(s == ES - 1))
                for s in range(ES):
                    sq_s = tmp.tile([P, w], BF16, tag="sq_s")
                    nc.vector.tensor_mul(sq_s, src_sb[:, s, sl],
                                         src_sb[:, s, sl])
                    nc.tensor.matmul(psum_q, ones_bf, sq_s,
                                     start=(s == 0), stop=(s == ES - 1))
                mu = tmp.tile([1, w], F32, tag="mu")
                nc.vector.tensor_scalar_mul(mu, psum_s, 1.0 / E)
                m2 = tmp.tile([1, w], F32, tag="m2")
                nc.vector.tensor_scalar_mul(m2, psum_q, 1.0 / E)
                var = tmp.tile([1, w], F32, tag="var")
                nc.vector.tensor_mul(var, mu, mu)
                nc.vector.tensor_sub(var, m2, var)
                nc.vector.tensor_scalar_add(var, var, EPS)
                # r = rsqrt(var) via reciprocal seed + 3 Newton steps (DVE
                # only -- avoids the ACT Sqrt table). var ~ 1 for LN inputs.
                w_ = tmp.tile([1, w], F32, tag="wrec")
                nc.vector.reciprocal(w_, var)
                r_ = tmp.tile([1, w], F32, tag="rr")
                nc.vector.tensor_scalar(r_, w_, 0.5, 0.5,
                                        mybir.AluOpType.mult,
                                        mybir.AluOpType.add)
                t_ = tmp.tile([1, w], F32, tag="tt")
                for _ in range(3):
                    nc.vector.tensor_mul(t_, r_, r_)
                    nc.vector.tensor_mul(t_, t_, var)
                    nc.vector.tensor_scalar(t_, t_, -0.5, 1.5,
                                            mybir.AluOpType.mult,
                                            mybir.AluOpType.add)
                    nc.vector.tensor_mul(r_, r_, t_)
                irow = tmp.tile([1, w], F32, tag="irow")
                nc.vector.tensor_mul(irow, var, r_)
                # a_bcast row 0 (cast to bf16), then broadcast to rows 1..127
                nc.vector.tensor_copy(a_bcast[0:1, sl], r_)
                # foldrow row0 = -mu (partition 0 -> 0, direct DVE)
                nc.vector.tensor_scalar_mul(foldrow[0:1, sl], mu, -1.0)
                # foldrow row1 = irow (partition 0 -> 1 via DMA)
                nc.gpsimd.dma_start(foldrow[1:2, sl], irow)
            bcast_rows(a_bcast[1:P, :], a_bcast[0:1, :], P - 1, ntok)
            if a_colT is not None:
                row_d = dpool.tile([1, ntok], BF16, tag="row_d")
                nc.sync.dma_start(row_d, a_bcast[0:1, :])
                nc.gpsimd.dma_start(
                    a_colT, row_d[0].rearrange("(t p) -> p t", p=P))

        with tc.tile_pool(name="att", bufs=1) as att:
            mask_sb = att.tile([P, KT, TQ], BF16, tag="mask")
            for kt in range(KT):
                nc.sync.dma_start(mask_sb[:, kt], mask3[:, kt])
            kf_sb = att.tile([P, E], BF16, tag="kf")
            nc.sync.dma_start(kf_sb, kf_d[:, :])
            vf_sb = att.tile([P, E], BF16, tag="vf")
            nc.sync.dma_start(vf_sb, vf_d[:, :])
            xkv_sb = att.tile([P, ES, TKV], BF16, tag="xkv")
            for s in range(ES):
                nc.sync.dma_start(xkv_sb[:, s], xkv3[:, s])
            foldrow_kv = att.tile([P, TKV], BF16, tag="foldrow_kv")
            nc.vector.memset(foldrow_kv, 0.0)
            a_kv = att.tile([P, TKV], BF16, tag="a_kv")
            a_colT = att.tile([P, KT], F32, tag="a_colT")
            ln_stats(xkv_sb, TKV, foldrow_kv, a_kv, a_colT)
            qt_all = att.tile([P, HPAIRS, TQ], BF16, tag="qt_all")

            # ---------- Q projection (all heads up front) ----------
            with tc.tile_pool(name="qproj", bufs=1) as qpool:
                xq_sb = qpool.tile([P, ES, TQ], BF16, tag="xq")
                for s in range(ES):
                    nc.sync.dma_start(xq_sb[:, s], xq3[:, s])
                qf_sb = qpool.tile([P, E], BF16, tag="qf")
                nc.sync.dma_start(qf_sb, qf_d[:, :])
                foldrow_q = qpool.tile([P, TQ], BF16, tag="foldrow_q")
                nc.vector.memset(foldrow_q, 0.0)
                a_q = qpool.tile([P, TQ], BF16, tag="a_q")
                ln_stats(xq_sb, TQ, foldrow_q, a_q)

                wq3 = wq_d.rearrange("(s p) d -> p s d", p=P)
                for m in range(HPAIRS):
                    wq_m = wstream.tile([P, ES, P], BF16, tag="w")
                    nc.sync.dma_start(wq_m, wq3[:, :, m * P:(m + 1) * P])
                    pst = ps.tile([P, 1024], F32, tag="ps")
                    for c in range(QC):
                        sl = slice(c * QW, (c + 1) * QW)
                        psl = pst[:, c * 512:c * 512 + QW]
                        for s in range(ES):
                            nc.tensor.matmul(psl, wq_m[:, s], xq_sb[:, s, sl],
                                             start=(s == 0), stop=False)
                        nc.tensor.matmul(psl, qf_sb[:, m * P:(m + 1) * P],
                                         foldrow_q[:, sl],
                                         start=False, stop=True)
                    nc.vector.tensor_tensor(qt_all[:, m], pst[:, 0:TQ], a_q,
                                            mybir.AluOpType.mult)

            # ---------- per-quad: K/V projections + attention ----------
            wk3 = wk_d.rearrange("(s p) d -> p s d", p=P)
            wv3 = wv_d.rearrange("(s p) d -> p s d", p=P)
            with tc.tile_pool(name="quad", bufs=2) as quad, \
                 tc.tile_pool(name="ppool", bufs=4) as ppool:
                for q4 in range(NQUAD):
                    kts = []
                    for pr2 in range(2):
                        m = q4 * 2 + pr2
                        wk_m = wstream.tile([P, ES, P], BF16, tag="w")
                        nc.sync.dma_start(wk_m, wk3[:, :, m * P:(m + 1) * P])
                        ktp = quad.tile([P, TKV], BF16, tag="ktp")
                        for g in range(_ceil_div(KVC, 2)):
                            pst = ps.tile([P, 1024], F32, tag="ps")
                            nh = min(2, KVC - g * 2)
                            for half in range(nh):
                                c = g * 2 + half
                                sl = slice(c * KVW, (c + 1) * KVW)
                                psl = pst[:, half * 512:half * 512 + KVW]
                                for s in range(ES):
                                    nc.tensor.matmul(
                                        psl, wk_m[:, s], xkv_sb[:, s, sl],
                                        start=(s == 0), stop=False)
                                nc.tensor.matmul(
                                    psl, kf_sb[:, m * P:(m + 1) * P],
                                    foldrow_kv[:, sl], start=False, stop=True)
                            w2g = nh * KVW
                            sl2 = slice(g * 2 * KVW, g * 2 * KVW + w2g)
                            nc.vector.tensor_tensor(
                                ktp[:, sl2], pst[:, 0:w2g], a_kv[:, sl2],
                                mybir.AluOpType.mult)
                        kts.append(ktp)

                    # V projection (token-major, 4 heads, +ones col)
                    vq = quad.tile([P, KT, 4 * 65], BF16, tag="vq")
                    wv_q = wstream.tile([P, ES, 4 * D], BF16, tag="w")
                    nc.sync.dma_start(
                        wv_q, wv3[:, :, q4 * 4 * D:(q4 + 1) * 4 * D])
                    vq_v = vq.rearrange("p t (h c) -> p t h c", c=65)
                    nc.vector.memset(vq_v[:, :, :, 64:65], 1.0)
                    for g in range(_ceil_div(KT, 2)):
                        pst = ps.tile([P, 1024], F32, tag="ps")
                        for half in range(min(2, KT - g * 2)):
                            tt = g * 2 + half
                            tsl = slice(tt * P, (tt + 1) * P)
                            psl = pst[:, half * 512:half * 512 + 4 * D]
                            for s in range(ES):
                                nc.tensor.matmul(psl, xkv_sb[:, s, tsl],
                                                 wv_q[:, s], start=(s == 0),
                                                 stop=False)
                            nc.tensor.matmul(
                                psl, foldrow_kv[:, tsl],
                                vf_sb[:, q4 * 4 * D:(q4 + 1) * 4 * D],
                                start=False, stop=True)
                            nc.vector.tensor_scalar_mul(
                                vq_v[:, tt, :, 0:64],
                                psl.rearrange("p (h c) -> p h c", c=64),
                                a_colT[:, tt:tt + 1])

                    # attention for the quad's two pairs
                    for pr2 in range(2):
                        m = q4 * 2 + pr2
                        ktp = kts[pr2]
                        opsA = pso.tile([65, 1024], F32, tag="opsum")
                        opsB = pso.tile([65, 1024], F32, tag="opsum")
                        for kt in range(KT):
                            ksl = slice(kt * P, (kt + 1) * P)
                            for hh, ops in ((0, opsA), (1, opsB)):
                                rows = slice(hh * 64, hh * 64 + 64)
                                sc = ps.tile([P, 1024], F32, tag="ps")
                                for c in range(QC):
                                    qsl = slice(c * QW, (c + 1) * QW)
                                    nc.tensor.matmul(
                                        sc[:, c * 512:c * 512 + QW],
                                        ktp[rows, ksl],
                                        qt_all[rows, m, qsl],
                                        start=True, stop=True)
                                pt = ppool.tile([P, 1024], BF16, tag="pT")
                                _ei = nc.scalar.activation(
                                    pt[:, 0:TQ], sc[:, 0:TQ], AF.Exp)
                                act_exp_insts.append(_ei)
                                add_dep_helper(
                                    _ei.ins, dummy_exp.ins, sync=True,
                                    reason="act table: exp after switch")
                                nc.vector.tensor_tensor(
                                    pt[:, 0:TQ], pt[:, 0:TQ], mask_sb[:, kt],
                                    mybir.AluOpType.mult)
                                h4 = 2 * pr2 + hh
                                vcols = slice(h4 * 65, h4 * 65 + 65)
                                for c in range(QC):
                                    nc.tensor.matmul(
                                        ops[:, c * 512:c * 512 + QW],
                                        vq[:, kt, vcols],
                                        pt[:, c * 512:c * 512 + QW],
                                        start=(kt == 0), stop=(kt == KT - 1))
                        # normalize: hidden = O / sum (sum at psum row 64)
                        for hh, ops in ((0, opsA), (1, opsB)):
                            ssb = tmp.tile([65, TQ], F32, tag="ssb")
                            nc.vector.reciprocal(ssb[64:65], ops[64:65, 0:TQ])
                            rb = tmp.tile([64, TQ], F32, tag="t4")
                            bcast_rows(rb, ssb[64:65, :], 64, TQ)
                            if hh == 0:
                                nc.vector.tensor_tensor(
                                    hidden[0:64, m], ops[0:64, 0:TQ], rb,
                                    mybir.AluOpType.mult)
                            else:
                                hb = tmp.tile([64, TQ], BF16, tag="hb")
                                nc.vector.tensor_tensor(
                                    hb, ops[0:64, 0:TQ], rb,
                                    mybir.AluOpType.mult)
                                nc.gpsimd.dma_start(hidden[64:128, m], hb)

        # ---------- Wo + residual, LN2, FFN ----------
        with tc.tile_pool(name="post", bufs=1) as post:
            out1 = post.tile([P, ES, TQ], F32, tag="out1")
            out1bf = post.tile([P, ES, TQ], BF16, tag="out1bf")
            wo3 = wo_d.rearrange("(s p) e -> p s e", p=P)
            for et in range(ES):
                wo_et = wstream.tile([P, ES, P], BF16, tag="w")
                nc.sync.dma_start(wo_et, wo3[:, :, et * P:(et + 1) * P])
                pst = ps.tile([P, 1024], F32, tag="ps")
                for c in range(QC):
                    psl = pst[:, c * 512:c * 512 + QW]
                    qsl = slice(c * QW, (c + 1) * QW)
                    for s in range(ES):
                        nc.tensor.matmul(psl, wo_et[:, s], hidden[:, s, qsl],
                                         start=(s == 0), stop=(s == ES - 1))
                xr = tmp.tile([P, TQ], F32, tag="t4")
                nc.sync.dma_start(xr, xqres3[:, et])
                nc.vector.tensor_add(out1[:, et], pst[:, 0:TQ], xr)
                nc.vector.tensor_copy(out1bf[:, et], out1[:, et])

            foldrow2 = post.tile([P, TQ], BF16, tag="foldrow2")
            nc.vector.memset(foldrow2, 0.0)
            a2 = post.tile([P, TQ], BF16, tag="a2")
            ln_stats(out1bf, TQ, foldrow2, a2)

            scr_out2 = perm.tile([1, 8], F32, tag="scr_out2")
            dummy_gelu = nc.scalar.activation(scr_out2, scr_in, AF.Gelu)
            for ei in act_exp_insts:
                add_dep_helper(dummy_gelu.ins, ei.ins, sync=True,
                               reason="act table: gelu after all exps")

            h3 = post.tile([P, FS, TQ], BF16, tag="h3")
            w1f_sb = post.tile([P, FF], BF16, tag="w1f")
            nc.sync.dma_start(w1f_sb, w1f_d[:, :])
            w13 = w1_d.rearrange("(s p) f -> p s f", p=P)
            for ft in range(FS):
                w1_ft = wstream.tile([P, ES, P], BF16, tag="w")
                nc.sync.dma_start(w1_ft, w13[:, :, ft * P:(ft + 1) * P])
                pst = ps.tile([P, 1024], F32, tag="ps")
                for c in range(QC):
                    psl = pst[:, c * 512:c * 512 + QW]
                    qsl = slice(c * QW, (c + 1) * QW)
                    for s in range(ES):
                        nc.tensor.matmul(psl, w1_ft[:, s], out1bf[:, s, qsl],
                                         start=(s == 0), stop=False)
                    nc.tensor.matmul(psl, w1f_sb[:, ft * P:(ft + 1) * P],
                                     foldrow2[:, qsl], start=False, stop=True)
                mid = tmp.tile([P, TQ], F32, tag="t4")
                nc.vector.tensor_tensor(mid, pst[:, 0:TQ], a2,
                                        mybir.AluOpType.mult)
                gi = nc.scalar.activation(h3[:, ft], mid, AF.Gelu,
                                          bias=b1f_sb[:, ft:ft + 1])
                add_dep_helper(gi.ins, dummy_gelu.ins, sync=True,
                               reason="act table: gelu after switch")

            w23 = w2_d.rearrange("(s p) e -> p s e", p=P)
            for et in range(ES):
                w2_et = wstream.tile([P, FS, P], BF16, tag="w")
                nc.sync.dma_start(w2_et, w23[:, :, et * P:(et + 1) * P])
                pst = ps.tile([P, 1024], F32, tag="ps")
                for c in range(QC):
                    psl = pst[:, c * 512:c * 512 + QW]
                    qsl = slice(c * QW, (c + 1) * QW)
                    for s in range(FS):
                        nc.tensor.matmul(psl, w2_et[:, s], h3[:, s, qsl],
                                         start=(s == 0), stop=(s == FS - 1))
                ot = tmp.tile([P, TQ], F32, tag="t4")
                nc.vector.scalar_tensor_tensor(
                    ot, pst[:, 0:TQ], b2f_sb[:, et:et + 1], out1[:, et],
                    mybir.AluOpType.add, mybir.AluOpType.add)
                nc.sync.dma_start(out3[:, et], ot)

    nc.compile()
    return nc


# ---------------------------------------------------------------------------
# Host side
# ---------------------------------------------------------------------------

def prep_inputs(dims, x, ln1_g, ln1_b, Wq, Wk, Wv, Wo, ln2_g, ln2_b,
                W1, b1, W2, b2):
    """Build per-core in_maps (list of dicts keyed by dram tensor names)."""
    B, C, E, H, D, FF = (dims["B"], dims["C"], dims["E"], dims["H"],
                         dims["D"], dims["FF"])
    TQ = B * C // N_CORES
    bf = ml_dtypes.bfloat16
    f32 = np.float32

    x = np.asarray(x, f32)
    sc = 1.0 / math.sqrt(D)
    wq = (ln1_g[:, None] * np.asarray(Wq, f32)) * sc
    bq = (ln1_b @ np.asarray(Wq, f32)) * sc
    wk = ln1_g[:, None] * np.asarray(Wk, f32)
    bk = ln1_b @ np.asarray(Wk, f32)
    wv = ln1_g[:, None] * np.asarray(Wv, f32)
    bv = ln1_b @ np.asarray(Wv, f32)
    w1 = ln2_g[:, None] * np.asarray(W1, f32)
    b1f = np.asarray(b1, f32) + ln2_b @ np.asarray(W1, f32)

    def fold(w, bias):
        f = np.zeros((P, w.shape[1]), f32)
        f[0] = w.sum(axis=0)
        f[1] = bias
        return f.astype(bf)

    shared = {
        "wq": wq.astype(bf), "wk": wk.astype(bf), "wv": wv.astype(bf),
        "wo": np.asarray(Wo, f32).astype(bf),
        "w1": w1.astype(bf), "w2": np.asarray(W2, f32).astype(bf),
        "qfold": fold(wq, bq), "kfold": fold(wk, bk), "vfold": fold(wv, bv),
        "w1fold": fold(w1, np.zeros(FF, f32)),
        "b1f": np.ascontiguousarray(b1f.reshape(FF // P, P).T),
        "b2f": np.ascontiguousarray(np.asarray(b2, f32).reshape(E // P, P).T),
    }

    nhalf = C // TQ  # query shards per batch
    in_maps = []
    for c in range(N_CORES):
        b = c // nhalf
        off = (c % nhalf) * TQ
        xb = x[b]                              # [C, E]
        xqf = xb[off:off + TQ]                 # [TQ, E]
        kpos = np.arange(C)[:, None]
        qpos = np.arange(TQ)[None, :] + off
        m = {
            "xkv": np.ascontiguousarray(xb.T).astype(bf),
            "xq": np.ascontiguousarray(xqf.T).astype(bf),
            "xqres": np.ascontiguousarray(xqf.T),
            "maskT": (kpos <= qpos).astype(bf),
        }
        m.update(shared)
        in_maps.append(m)
    return in_maps


def assemble_output(dims, results):
    B, C, E = dims["B"], dims["C"], dims["E"]
    TQ = B * C // N_CORES
    nhalf = C // TQ
    out = np.empty((B, C, E), np.float32)
    for c in range(N_CORES):
        b = c // nhalf
        off = (c % nhalf) * TQ
        out[b, off:off + TQ] = results[c]["outT"].T
    return out


def kernel(**inputs):
    dims = DIMS
    nc = build_program(dims)
    in_maps = prep_inputs(dims, **{k: np.asarray(v) for k, v in
                                   inputs.items()})
    res = run_bass_kernel_spmd(nc, in_maps, list(range(N_CORES)))
    return assemble_output(dims, res.results)


if __name__ == "__main__":
    nc = build_program(DIMS)
    print("build ok")


# revision 20
# speedup vs baseline: 1.1915x; 1.1915x over previous
"""Trainium2 Bass kernel for a pre-LN causal transformer block.

Sharding: data-parallel over (batch, sequence-half) -> 8 uniform SPMD shards.
Each core handles 1024 queries of one batch against that batch's 2048 keys,
with causality enforced by a host-supplied multiplicative mask applied after
exp (so the SPMD program is identical on every core).

Layout: all activations are E-major ("transposed", [E, tokens]) so every
matmul contraction lands on the partition dim with zero on-device transposes.
LayerNorm is folded into the projections:
    h = (x - mu) * r * gamma + beta;  h @ W
      = r_t * (x @ (gamma*W)) + (-mu_t) * colsum(gamma*W) + beta @ W
so the device computes raw = (gamma*W)^T @ xT, adds the rank-2 correction
( rows (-mu, 1/r) x rows (colsum, beta@W) ) via a K=128-padded matmul into
PSUM, and multiplies by r_t during the PSUM->SBUF copy.

Matmul inputs are bf16 (full PE rate), accumulation f32 in PSUM, softmax and
residuals f32.
"""

import math
from contextlib import ExitStack

import numpy as np
import ml_dtypes

import concourse.bass as bass
import concourse.tile as tile
from concourse import bacc
from concourse.tile import add_dep_helper
from concourse import mybir
from concourse.bass_utils import run_bass_kernel_spmd

F32 = mybir.dt.float32
BF16 = mybir.dt.bfloat16
AF = mybir.ActivationFunctionType

# Full-size problem dims (hardcoded; the harness provides x of this shape).
DIMS = dict(B=4, C=2048, E=1024, H=16, D=64, FF=4096, EPS=1e-5)
N_CORES = 8
P = 128


def _ceil_div(a, b):
    return (a + b - 1) // b


def coalesce_sem_updates(nc):
    """Drop sem increments whose cumulative value no wait references,
    folding their count into the next surviving increment on the same
    engine stream. Sound: every waited-on satisfaction point keeps its
    original cumulative value and firing position; only unobserved
    intermediate values are delayed. Motivated by this environment's
    ~5-8us cost per semaphore op."""
    import bass_rust
    insts = [i for bb in nc.m.functions[0].blocks for i in bb.instructions]
    waited = {}
    for i in insts:
        si = getattr(i, "sync_info", None)
        if si is None:
            continue
        for w in si.on_wait:
            ok = (w.sync_type == "semaphore" and w.wait_reg is None
                  and w.wait_mode == "sem-ge-imm")
            waited.setdefault(w.id, set()).add(w.wait_value if ok else None)
    writers = {}
    poison = set()
    for i in insts:
        si = getattr(i, "sync_info", None)
        if si is None:
            continue
        is_dma = type(i).__name__ == "InstDMACopy"
        ekey = "DMA" if is_dma else str(getattr(i, "engine", None))
        for u in si.on_update:
            if (u.sync_type != "semaphore" or u.update_mode != "sem-inc"
                    or u.update_reg is not None or is_dma):
                poison.add(u.id)
            writers.setdefault(u.id, set()).add(ekey)
    for sid, ws in writers.items():
        if len(ws) > 1:
            poison.add(sid)
    for sid, vals in waited.items():
        if None in vals:
            poison.add(sid)

    # sem range-resets (Drain is_reset_sema / EVENT_SEMAPHORE_RANGE_CLEAR)
    # restart a sem's cumulative count; segment the walk at each one.
    resets = {}  # inst idx -> (first, last) sem id range cleared
    for idx, i in enumerate(insts):
        rs = getattr(i, "reset_range_start", None)
        re_ = getattr(i, "reset_range_stop", None)
        if getattr(i, "is_reset_sema", False) and rs is not None:
            resets[idx] = (rs, re_)
        rf = getattr(i, "range_first", None)
        rl = getattr(i, "range_last", None)
        if rf is not None and rl is not None:
            resets[idx] = (rf, rl + 1)

    # locate each sem's updates in stream order
    upd_sites = {}
    for idx, i in enumerate(insts):
        si = getattr(i, "sync_info", None)
        if si is None:
            continue
        for u in si.on_update:
            if u.id in poison or u.id not in writers:
                continue
            upd_sites.setdefault(u.id, []).append(idx)

    new_vals = {}   # (inst_idx, sem_id) -> new update_value (0 = drop)
    ndrop = 0
    for sid, sites in upd_sites.items():
        cut_at = sorted(idx for idx, (a, b) in resets.items()
                        if a <= sid < b)
        # split sites into segments between resets
        segments = []
        seg = []
        ci = 0
        for idx in sites:
            while ci < len(cut_at) and cut_at[ci] < idx:
                if seg:
                    segments.append(seg)
                    seg = []
                ci += 1
            seg.append(idx)
        if seg:
            segments.append(seg)
        wvals = sorted(v for v in waited.get(sid, set()) if v is not None)
        for seg in segments:
            # only touch segments bounded by a reset on BOTH sides (the main
            # body); preamble/tail stream semantics are left untouched.
            if not (cut_at and cut_at[0] < seg[0] and cut_at[-1] > seg[-1]):
                continue
            cum = 0
            last_kept_cum = 0
            for pos, idx in enumerate(seg):
                si = insts[idx].sync_info
                uval = next(u.update_value for u in si.on_update
                            if u.id == sid)
                cum += uval
                referenced = any(last_kept_cum < w <= cum for w in wvals)
                if referenced or pos == len(seg) - 1:
                    new_vals[(idx, sid)] = cum - last_kept_cum
                    last_kept_cum = cum
                else:
                    new_vals[(idx, sid)] = 0
                    ndrop += 1

    for idx, i in enumerate(insts):
        si = getattr(i, "sync_info", None)
        if si is None:
            continue
        touched = any((idx, u.id) in new_vals for u in si.on_update)
        if not touched:
            continue
        keep = []
        for u in si.on_update:
            nv = new_vals.get((idx, u.id))
            if nv is None:
                keep.append(u)
            elif nv > 0:
                u.update_value = nv
                keep.append(u)
        i.sync_info = bass_rust.SyncInfo(on_wait=list(si.on_wait),
                                         on_update=keep)
    return ndrop


def build_program(dims):
    """Build the SPMD Bass program. Returns nc."""
    B = dims["B"]
    C = dims["C"]
    E = dims["E"]
    H = dims["H"]
    D = dims["D"]
    FF = dims["FF"]
    EPS = dims["EPS"]

    TKV = C                      # kv tokens per core
    TQ = B * C // N_CORES        # query tokens per core
    ES = E // P                  # E subtiles (contraction)
    FS = FF // P                 # FF subtiles
    HPAIRS = H // 2              # head pairs (Qt/Kt partition packing)
    NQUAD = H // 4               # head quads (V projection batches)
    KT = TKV // P                # key tiles
    QTT = TQ // P                # query token tiles
    QC = _ceil_div(TQ, 512)      # 512-wide query chunks
    QW = TQ // QC                # query chunk width (512 normally)
    KVC = _ceil_div(TKV, 512)    # 512-wide kv chunks
    KVW = TKV // KVC
    assert D == 64 and E == H * D

    nc = bacc.Bacc("TRN2", target_bir_lowering=False, debug=False)

    # ---- DRAM I/O ----
    xkv_d = nc.dram_tensor("xkv", [E, TKV], BF16, kind="ExternalInput")
    xq_d = nc.dram_tensor("xq", [E, TQ], BF16, kind="ExternalInput")
    xqres_d = nc.dram_tensor("xqres", [E, TQ], F32, kind="ExternalInput")
    mask_d = nc.dram_tensor("maskT", [TKV, TQ], BF16, kind="ExternalInput")
    wq_d = nc.dram_tensor("wq", [E, E], BF16, kind="ExternalInput")
    wk_d = nc.dram_tensor("wk", [E, E], BF16, kind="ExternalInput")
    wv_d = nc.dram_tensor("wv", [E, E], BF16, kind="ExternalInput")
    wo_d = nc.dram_tensor("wo", [E, E], BF16, kind="ExternalInput")
    w1_d = nc.dram_tensor("w1", [E, FF], BF16, kind="ExternalInput")
    w2_d = nc.dram_tensor("w2", [FF, E], BF16, kind="ExternalInput")
    # fold tensors: row0 = colsum(W'), row1 = beta @ W  (padded to 128 rows)
    qf_d = nc.dram_tensor("qfold", [P, E], BF16, kind="ExternalInput")
    kf_d = nc.dram_tensor("kfold", [P, E], BF16, kind="ExternalInput")
    vf_d = nc.dram_tensor("vfold", [P, E], BF16, kind="ExternalInput")
    w1f_d = nc.dram_tensor("w1fold", [P, FF], BF16, kind="ExternalInput")
    b1f_d = nc.dram_tensor("b1f", [P, FS], F32, kind="ExternalInput")
    b2f_d = nc.dram_tensor("b2f", [P, ES], F32, kind="ExternalInput")
    out_d = nc.dram_tensor("outT", [E, TQ], F32, kind="ExternalOutput")

    xkv3 = xkv_d.rearrange("(s p) t -> p s t", p=P)
    xq3 = xq_d.rearrange("(s p) t -> p s t", p=P)
    xqres3 = xqres_d.rearrange("(s p) t -> p s t", p=P)
    mask3 = mask_d.rearrange("(s p) t -> p s t", p=P)
    out3 = out_d.rearrange("(s p) t -> p s t", p=P)

    with tile.TileContext(nc) as tc, ExitStack() as ctx:
        perm = ctx.enter_context(tc.tile_pool(name="perm", bufs=1))
        tmp = ctx.enter_context(tc.tile_pool(name="tmp", bufs=2))
        wstream = ctx.enter_context(tc.tile_pool(name="wstream", bufs=2))
        ps = ctx.enter_context(tc.tile_pool(name="ps", bufs=2, space="PSUM"))
        pso = ctx.enter_context(tc.tile_pool(name="pso", bufs=2, space="PSUM"))
        dpool = ctx.enter_context(tc.tile_pool(name="dpool", bufs=2,
                                               space="DRAM"))

        def bcast_rows(dst, srcrow, nrows, width):
            """Broadcast a [1, width] sbuf row to [nrows, width] via DRAM."""
            row_d = dpool.tile([1, width], srcrow.dtype, tag="row_d")
            nc.sync.dma_start(row_d, srcrow)
            bsrc = bass.AP(tensor=row_d.tensor, offset=row_d.offset,
                           ap=[[0, nrows]] + row_d.ap[1:])
            nc.gpsimd.dma_start(dst, bsrc)

        ones_bf = perm.tile([P, 1], BF16, tag="ones_bf")
        nc.vector.memset(ones_bf, 1.0)

        # ACT LUT table management: Exp and Gelu live in different hardware
        # tables, and walrus's table-switch rides the first activation using
        # the new table -- which then only supports a single sync wait. Emit
        # zero-dependency dummy activations to carry each switch; order them
        # on the ACT stream with same-engine dep edges (no semaphores).
        scr_in = perm.tile([1, 8], F32, tag="scr_in")
        nc.vector.memset(scr_in, 1.0)
        scr_out = perm.tile([1, 8], F32, tag="scr_out")
        dummy_exp = nc.scalar.activation(scr_out, scr_in, AF.Exp)
        act_exp_insts = []

        # Warm up every DVE / PE opcode on scratch so first-use config
        # loads don't ride real (multi-wait) instructions.
        A = mybir.AluOpType
        nc.vector.tensor_copy(scr_out, scr_in)
        nc.vector.tensor_mul(scr_out, scr_in, scr_in)
        nc.vector.tensor_add(scr_out, scr_in, scr_in)
        nc.vector.tensor_sub(scr_out, scr_in, scr_in)
        nc.vector.tensor_scalar(scr_out, scr_in, 0.5, 0.5, A.mult, A.add)
        nc.vector.tensor_scalar_mul(scr_out, scr_in, 0.5)
        nc.vector.tensor_scalar_add(scr_out, scr_in, 0.5)
        nc.vector.reciprocal(scr_out, scr_in)
        nc.vector.scalar_tensor_tensor(scr_out, scr_in, 0.5, scr_in,
                                       A.add, A.add)
        scr_bf = perm.tile([1, 8], BF16, tag="scr_bf")
        nc.vector.memset(scr_bf, 1.0)
        nc.vector.tensor_mul(scr_bf, scr_bf, scr_bf)
        scr_ps = ps.tile([P, 1024], F32, tag="ps")
        nc.tensor.matmul(scr_ps[0:8, 0:8], scr_bf[0:1, 0:8],
                         scr_bf[0:1, 0:8], start=True, stop=True)
        nc.vector.tensor_copy(scr_out, scr_ps[0:1, 0:8])
        b1f_sb = perm.tile([P, FS], F32, tag="b1f")
        nc.sync.dma_start(b1f_sb, b1f_d[:, :])
        b2f_sb = perm.tile([P, ES], F32, tag="b2f")
        nc.sync.dma_start(b2f_sb, b2f_d[:, :])
        hidden = perm.tile([P, HPAIRS, TQ], BF16, tag="hidden")

        # ---------- LN statistics (per token, over E) ----------
        # foldrow[0] = -mu, foldrow[1] = 1/r = sqrt(var+eps); rows 2.. = 0.
        # a_bcast = r broadcast to all 128 partitions (bf16).
        def ln_stats(src_sb, ntok, foldrow, a_bcast, a_colT=None):
            nchunk = _ceil_div(ntok, 512)
            w = ntok // nchunk
            for c in range(nchunk):
                sl = slice(c * w, (c + 1) * w)
                pst = ps.tile([P, 1024], F32, tag="ps")
                psum_s = pst[0:1, 0:w]
                psum_q = pst[0:1, 512:512 + w]
                for s in range(ES):
                    nc.tensor.matmul(psum_s, ones_bf, src_sb[:, s, sl],
                                     start=(s == 0), stop=(s == ES - 1))
                for s in range(ES):
                    sq_s = tmp.tile([P, w], BF16, tag="sq_s")
                    nc.vector.tensor_mul(sq_s, src_sb[:, s, sl],
                                         src_sb[:, s, sl])
                    nc.tensor.matmul(psum_q, ones_bf, sq_s,
                                     start=(s == 0), stop=(s == ES - 1))
                mu = tmp.tile([1, w], F32, tag="mu")
                nc.vector.tensor_scalar_mul(mu, psum_s, 1.0 / E)
                m2 = tmp.tile([1, w], F32, tag="m2")
                nc.vector.tensor_scalar_mul(m2, psum_q, 1.0 / E)
                var = tmp.tile([1, w], F32, tag="var")
                nc.vector.tensor_mul(var, mu, mu)
                nc.vector.tensor_sub(var, m2, var)
                nc.vector.tensor_scalar_add(var, var, EPS)
                # r = rsqrt(var) via reciprocal seed + 3 Newton steps (DVE
                # only -- avoids the ACT Sqrt table). var ~ 1 for LN inputs.
                w_ = tmp.tile([1, w], F32, tag="wrec")
                nc.vector.reciprocal(w_, var)
                r_ = tmp.tile([1, w], F32, tag="rr")
                nc.vector.tensor_scalar(r_, w_, 0.5, 0.5,
                                        mybir.AluOpType.mult,
                                        mybir.AluOpType.add)
                t_ = tmp.tile([1, w], F32, tag="tt")
                for _ in range(3):
                    nc.vector.tensor_mul(t_, r_, r_)
                    nc.vector.tensor_mul(t_, t_, var)
                    nc.vector.tensor_scalar(t_, t_, -0.5, 1.5,
                                            mybir.AluOpType.mult,
                                            mybir.AluOpType.add)
                    nc.vector.tensor_mul(r_, r_, t_)
                irow = tmp.tile([1, w], F32, tag="irow")
                nc.vector.tensor_mul(irow, var, r_)
                # a_bcast row 0 (cast to bf16), then broadcast to rows 1..127
                nc.vector.tensor_copy(a_bcast[0:1, sl], r_)
                # foldrow row0 = -mu (partition 0 -> 0, direct DVE)
                nc.vector.tensor_scalar_mul(foldrow[0:1, sl], mu, -1.0)
                # foldrow row1 = irow (partition 0 -> 1 via DMA)
                nc.gpsimd.dma_start(foldrow[1:2, sl], irow)
            bcast_rows(a_bcast[1:P, :], a_bcast[0:1, :], P - 1, ntok)
            if a_colT is not None:
                row_d = dpool.tile([1, ntok], BF16, tag="row_d")
                nc.sync.dma_start(row_d, a_bcast[0:1, :])
                nc.gpsimd.dma_start(
                    a_colT, row_d[0].rearrange("(t p) -> p t", p=P))

        with tc.tile_pool(name="att", bufs=1) as att:
            mask_sb = att.tile([P, KT, TQ], BF16, tag="mask")
            for kt in range(KT):
                nc.sync.dma_start(mask_sb[:, kt], mask3[:, kt])
            kf_sb = att.tile([P, E], BF16, tag="kf")
            nc.sync.dma_start(kf_sb, kf_d[:, :])
            vf_sb = att.tile([P, E], BF16, tag="vf")
            nc.sync.dma_start(vf_sb, vf_d[:, :])
            xkv_sb = att.tile([P, ES, TKV], BF16, tag="xkv")
            for s in range(ES):
                nc.sync.dma_start(xkv_sb[:, s], xkv3[:, s])
            foldrow_kv = att.tile([P, TKV], BF16, tag="foldrow_kv")
            nc.vector.memset(foldrow_kv, 0.0)
            a_kv = att.tile([P, TKV], BF16, tag="a_kv")
            a_colT = att.tile([P, KT], F32, tag="a_colT")
            ln_stats(xkv_sb, TKV, foldrow_kv, a_kv, a_colT)
            qt_all = att.tile([P, HPAIRS, TQ], BF16, tag="qt_all")

            # ---------- Q projection (all heads up front) ----------
            with tc.tile_pool(name="qproj", bufs=1) as qpool:
                xq_sb = qpool.tile([P, ES, TQ], BF16, tag="xq")
                for s in range(ES):
                    nc.sync.dma_start(xq_sb[:, s], xq3[:, s])
                qf_sb = qpool.tile([P, E], BF16, tag="qf")
                nc.sync.dma_start(qf_sb, qf_d[:, :])
                foldrow_q = qpool.tile([P, TQ], BF16, tag="foldrow_q")
                nc.vector.memset(foldrow_q, 0.0)
                a_q = qpool.tile([P, TQ], BF16, tag="a_q")
                ln_stats(xq_sb, TQ, foldrow_q, a_q)

                wq3 = wq_d.rearrange("(s p) d -> p s d", p=P)
                for m in range(HPAIRS):
                    wq_m = wstream.tile([P, ES, P], BF16, tag="w")
                    nc.sync.dma_start(wq_m, wq3[:, :, m * P:(m + 1) * P])
                    pst = ps.tile([P, 1024], F32, tag="ps")
                    for c in range(QC):
                        sl = slice(c * QW, (c + 1) * QW)
                        psl = pst[:, c * 512:c * 512 + QW]
                        for s in range(ES):
                            nc.tensor.matmul(psl, wq_m[:, s], xq_sb[:, s, sl],
                                             start=(s == 0), stop=False)
                        nc.tensor.matmul(psl, qf_sb[:, m * P:(m + 1) * P],
                                         foldrow_q[:, sl],
                                         start=False, stop=True)
                    nc.vector.tensor_tensor(qt_all[:, m], pst[:, 0:TQ], a_q,
                                            mybir.AluOpType.mult)

            # ---------- per-quad: K/V projections + attention ----------
            wk3 = wk_d.rearrange("(s p) d -> p s d", p=P)
            wv3 = wv_d.rearrange("(s p) d -> p s d", p=P)
            with tc.tile_pool(name="quad", bufs=2) as quad, \
                 tc.tile_pool(name="ppool", bufs=4) as ppool:
                for q4 in range(NQUAD):
                    kts = []
                    for pr2 in range(2):
                        m = q4 * 2 + pr2
                        wk_m = wstream.tile([P, ES, P], BF16, tag="w")
                        nc.sync.dma_start(wk_m, wk3[:, :, m * P:(m + 1) * P])
                        ktp = quad.tile([P, TKV], BF16, tag="ktp")
                        for g in range(_ceil_div(KVC, 2)):
                            pst = ps.tile([P, 1024], F32, tag="ps")
                            nh = min(2, KVC - g * 2)
                            for half in range(nh):
                                c = g * 2 + half
                                sl = slice(c * KVW, (c + 1) * KVW)
                                psl = pst[:, half * 512:half * 512 + KVW]
                                for s in range(ES):
                                    nc.tensor.matmul(
                                        psl, wk_m[:, s], xkv_sb[:, s, sl],
                                        start=(s == 0), stop=False)
                                nc.tensor.matmul(
                                    psl, kf_sb[:, m * P:(m + 1) * P],
                                    foldrow_kv[:, sl], start=False, stop=True)
                            w2g = nh * KVW
                            sl2 = slice(g * 2 * KVW, g * 2 * KVW + w2g)
                            nc.vector.tensor_tensor(
                                ktp[:, sl2], pst[:, 0:w2g], a_kv[:, sl2],
                                mybir.AluOpType.mult)
                        kts.append(ktp)

                    # V projection (token-major, 4 heads, +ones col)
                    vq = quad.tile([P, KT, 4 * 65], BF16, tag="vq")
                    wv_q = wstream.tile([P, ES, 4 * D], BF16, tag="w")
                    nc.sync.dma_start(
                        wv_q, wv3[:, :, q4 * 4 * D:(q4 + 1) * 4 * D])
                    vq_v = vq.rearrange("p t (h c) -> p t h c", c=65)
                    nc.vector.memset(vq_v[:, :, :, 64:65], 1.0)
                    for g in range(_ceil_div(KT, 2)):
                        pst = ps.tile([P, 1024], F32, tag="ps")
                        for half in range(min(2, KT - g * 2)):
                            tt = g * 2 + half
                            tsl = slice(tt * P, (tt + 1) * P)
                            psl = pst[:, half * 512:half * 512 + 4 * D]
                            for s in range(ES):
                                nc.tensor.matmul(psl, xkv_sb[:, s, tsl],
                                                 wv_q[:, s], start=(s == 0),
                                                 stop=False)
                            nc.tensor.matmul(
                                psl, foldrow_kv[:, tsl],
                                vf_sb[:, q4 * 4 * D:(q4 + 1) * 4 * D],
                                start=False, stop=True)
                            nc.vector.tensor_scalar_mul(
                                vq_v[:, tt, :, 0:64],
                                psl.rearrange("p (h c) -> p h c", c=64),
                                a_colT[:, tt:tt + 1])

                    # attention for the quad's two pairs
                    for pr2 in range(2):
                        m = q4 * 2 + pr2
                        ktp = kts[pr2]
                        opsA = pso.tile([65, 1024], F32, tag="opsum")
                        opsB = pso.tile([65, 1024], F32, tag="opsum")
                        for kt in range(KT):
                            ksl = slice(kt * P, (kt + 1) * P)
                            for hh, ops in ((0, opsA), (1, opsB)):
                                rows = slice(hh * 64, hh * 64 + 64)
                                sc = ps.tile([P, 1024], F32, tag="ps")
                                for c in range(QC):
                                    qsl = slice(c * QW, (c + 1) * QW)
                                    nc.tensor.matmul(
                                        sc[:, c * 512:c * 512 + QW],
                                        ktp[rows, ksl],
                                        qt_all[rows, m, qsl],
                                        start=True, stop=True)
                                pt = ppool.tile([P, 1024], BF16, tag="pT")
                                _ei = nc.scalar.activation(
                                    pt[:, 0:TQ], sc[:, 0:TQ], AF.Exp)
                                act_exp_insts.append(_ei)
                                add_dep_helper(
                                    _ei.ins, dummy_exp.ins, sync=True,
                                    reason="act table: exp after switch")
                                nc.vector.tensor_tensor(
                                    pt[:, 0:TQ], pt[:, 0:TQ], mask_sb[:, kt],
                                    mybir.AluOpType.mult)
                                h4 = 2 * pr2 + hh
                                vcols = slice(h4 * 65, h4 * 65 + 65)
                                for c in range(QC):
                                    nc.tensor.matmul(
                                        ops[:, c * 512:c * 512 + QW],
                                        vq[:, kt, vcols],
                                        pt[:, c * 512:c * 512 + QW],
                                        start=(kt == 0), stop=(kt == KT - 1))
                        # normalize: hidden = O / sum (sum at psum row 64)
                        for hh, ops in ((0, opsA), (1, opsB)):
                            ssb = tmp.tile([65, TQ], F32, tag="ssb")
                            nc.vector.reciprocal(ssb[64:65], ops[64:65, 0:TQ])
                            rb = tmp.tile([64, TQ], F32, tag="t4")
                            bcast_rows(rb, ssb[64:65, :], 64, TQ)
                            if hh == 0:
                                nc.vector.tensor_tensor(
                                    hidden[0:64, m], ops[0:64, 0:TQ], rb,
                                    mybir.AluOpType.mult)
                            else:
                                hb = tmp.tile([64, TQ], BF16, tag="hb")
                                nc.vector.tensor_tensor(
                                    hb, ops[0:64, 0:TQ], rb,
                                    mybir.AluOpType.mult)
                                nc.gpsimd.dma_start(hidden[64:128, m], hb)

        # ---------- Wo + residual, LN2, FFN ----------
        with tc.tile_pool(name="post", bufs=1) as post:
            out1 = post.tile([P, ES, TQ], F32, tag="out1")
            out1bf = post.tile([P, ES, TQ], BF16, tag="out1bf")
            wo3 = wo_d.rearrange("(s p) e -> p s e", p=P)
            for et in range(ES):
                wo_et = wstream.tile([P, ES, P], BF16, tag="w")
                nc.sync.dma_start(wo_et, wo3[:, :, et * P:(et + 1) * P])
                pst = ps.tile([P, 1024], F32, tag="ps")
                for c in range(QC):
                    psl = pst[:, c * 512:c * 512 + QW]
                    qsl = slice(c * QW, (c + 1) * QW)
                    for s in range(ES):
                        nc.tensor.matmul(psl, wo_et[:, s], hidden[:, s, qsl],
                                         start=(s == 0), stop=(s == ES - 1))
                xr = tmp.tile([P, TQ], F32, tag="t4")
                nc.sync.dma_start(xr, xqres3[:, et])
                nc.vector.tensor_add(out1[:, et], pst[:, 0:TQ], xr)
                nc.vector.tensor_copy(out1bf[:, et], out1[:, et])

            foldrow2 = post.tile([P, TQ], BF16, tag="foldrow2")
            nc.vector.memset(foldrow2, 0.0)
            a2 = post.tile([P, TQ], BF16, tag="a2")
            ln_stats(out1bf, TQ, foldrow2, a2)

            scr_out2 = perm.tile([1, 8], F32, tag="scr_out2")
            dummy_gelu = nc.scalar.activation(scr_out2, scr_in, AF.Gelu)
            for ei in act_exp_insts:
                add_dep_helper(dummy_gelu.ins, ei.ins, sync=True,
                               reason="act table: gelu after all exps")

            h3 = post.tile([P, FS, TQ], BF16, tag="h3")
            w1f_sb = post.tile([P, FF], BF16, tag="w1f")
            nc.sync.dma_start(w1f_sb, w1f_d[:, :])
            w13 = w1_d.rearrange("(s p) f -> p s f", p=P)
            for ft in range(FS):
                w1_ft = wstream.tile([P, ES, P], BF16, tag="w")
                nc.sync.dma_start(w1_ft, w13[:, :, ft * P:(ft + 1) * P])
                pst = ps.tile([P, 1024], F32, tag="ps")
                for c in range(QC):
                    psl = pst[:, c * 512:c * 512 + QW]
                    qsl = slice(c * QW, (c + 1) * QW)
                    for s in range(ES):
                        nc.tensor.matmul(psl, w1_ft[:, s], out1bf[:, s, qsl],
                                         start=(s == 0), stop=False)
                    nc.tensor.matmul(psl, w1f_sb[:, ft * P:(ft + 1) * P],
                                     foldrow2[:, qsl], start=False, stop=True)
                mid = tmp.tile([P, TQ], F32, tag="t4")
                nc.vector.tensor_tensor(mid, pst[:, 0:TQ], a2,
                                        mybir.AluOpType.mult)
                gi = nc.scalar.activation(h3[:, ft], mid, AF.Gelu,
                                          bias=b1f_sb[:, ft:ft + 1])
                add_dep_helper(gi.ins, dummy_gelu.ins, sync=True,
                               reason="act table: gelu after switch")

            w23 = w2_d.rearrange("(s p) e -> p s e", p=P)
            for et in range(ES):
                w2_et = wstream.tile([P, FS, P], BF16, tag="w")
                nc.sync.dma_start(w2_et, w23[:, :, et * P:(et + 1) * P])
                pst = ps.tile([P, 1024], F32, tag="ps")
                for c in range(QC):
                    psl = pst[:, c * 512:c * 512 + QW]
                    qsl = slice(c * QW, (c + 1) * QW)
                    for s in range(FS):
                        nc.tensor.matmul(psl, w2_et[:, s], h3[:, s, qsl],
                                         start=(s == 0), stop=(s == FS - 1))
                ot = tmp.tile([P, TQ], F32, tag="t4")
                nc.vector.scalar_tensor_tensor(
                    ot, pst[:, 0:TQ], b2f_sb[:, et:et + 1], out1[:, et],
                    mybir.AluOpType.add, mybir.AluOpType.add)
                nc.sync.dma_start(out3[:, et], ot)

    nc.compile()
    n = coalesce_sem_updates(nc)
    return nc


# ---------------------------------------------------------------------------
# Host side
# ---------------------------------------------------------------------------

def prep_inputs(dims, x, ln1_g, ln1_b, Wq, Wk, Wv, Wo, ln2_g, ln2_b,
                W1, b1, W2, b2):
    """Build per-core in_maps (list of dicts keyed by dram tensor names)."""
    B, C, E, H, D, FF = (dims["B"], dims["C"], dims["E"], dims["H"],
                         dims["D"], dims["FF"])
    TQ = B * C // N_CORES
    bf = ml_dtypes.bfloat16
    f32 = np.float32

    x = np.asarray(x, f32)
    sc = 1.0 / math.sqrt(D)
    wq = (ln1_g[:, None] * np.asarray(Wq, f32)) * sc
    bq = (ln1_b @ np.asarray(Wq, f32)) * sc
    wk = ln1_g[:, None] * np.asarray(Wk, f32)
    bk = ln1_b @ np.asarray(Wk, f32)
    wv = ln1_g[:, None] * np.asarray(Wv, f32)
    bv = ln1_b @ np.asarray(Wv, f32)
    w1 = ln2_g[:, None] * np.asarray(W1, f32)
    b1f = np.asarray(b1, f32) + ln2_b @ np.asarray(W1, f32)

    def fold(w, bias):
        f = np.zeros((P, w.shape[1]), f32)
        f[0] = w.sum(axis=0)
        f[1] = bias
        return f.astype(bf)

    shared = {
        "wq": wq.astype(bf), "wk": wk.astype(bf), "wv": wv.astype(bf),
        "wo": np.asarray(Wo, f32).astype(bf),
        "w1": w1.astype(bf), "w2": np.asarray(W2, f32).astype(bf),
        "qfold": fold(wq, bq), "kfold": fold(wk, bk), "vfold": fold(wv, bv),
        "w1fold": fold(w1, np.zeros(FF, f32)),
        "b1f": np.ascontiguousarray(b1f.reshape(FF // P, P).T),
        "b2f": np.ascontiguousarray(np.asarray(b2, f32).reshape(E // P, P).T),
    }

    nhalf = C // TQ  # query shards per batch
    in_maps = []
    for c in range(N_CORES):
        b = c // nhalf
        off = (c % nhalf) * TQ
        xb = x[b]                              # [C, E]
        xqf = xb[off:off + TQ]                 # [TQ, E]
        kpos = np.arange(C)[:, None]
        qpos = np.arange(TQ)[None, :] + off
        m = {
            "xkv": np.ascontiguousarray(xb.T).astype(bf),
            "xq": np.ascontiguousarray(xqf.T).astype(bf),
            "xqres": np.ascontiguousarray(xqf.T),
            "maskT": (kpos <= qpos).astype(bf),
        }
        m.update(shared)
        in_maps.append(m)
    return in_maps


def assemble_output(dims, results):
    B, C, E = dims["B"], dims["C"], dims["E"]
    TQ = B * C // N_CORES
    nhalf = C // TQ
    out = np.empty((B, C, E), np.float32)
    for c in range(N_CORES):
        b = c // nhalf
        off = (c % nhalf) * TQ
        out[b, off:off + TQ] = results[c]["outT"].T
    return out


def kernel(**inputs):
    dims = DIMS
    nc = build_program(dims)
    in_maps = prep_inputs(dims, **{k: np.asarray(v) for k, v in
                                   inputs.items()})
    res = run_bass_kernel_spmd(nc, in_maps, list(range(N_CORES)))
    return assemble_output(dims, res.results)


if __name__ == "__main__":
    nc = build_program(DIMS)
    print("build ok")
